# revision 34
# baseline (speedup 1.0000x reference)
"""Trainium2 Bass kernel for the DimeNet-style directed-message block.

Reference computation (W = n_angles, E = n_edges, D = 128, A = 49, J = 8):
    m_kj     = m_ji[kj_idx]                          # [W, D]
    transf_m = silu(m_kj @ W_nbr + b_nbr)            # [W, D]
    transf_e = e_rbf[kj_idx] @ W_e                   # [W, D]
    m_and_e  = transf_m * transf_e                   # [W, D]
    transf_a = a_sbf @ W_a                           # [W, J]
    out[w,i] = sum_{j,l} transf_a[w,j] m_and_e[w,l] final_w[i,j,l]
    final    = segment_sum(out, kj_idx, E)           # [E, D]

Algebraic refactor: the segment sum commutes through the bilinear form:
    me       = silu(m_ji @ W_nbr + b) * (e_rbf @ W_e)        # [E, D]
    S        = segment_sum(a_sbf @ W_a, kj_idx, E)           # [E, J]
    final[e] = sum_j S[e,j] * (me[e] @ final_w[:,j,:].T)     # [E, D]

S without scatter: edges are sharded contiguously (25000/core, angles
binned by owner core kj // 25000) and permuted within the core by
descending angle multiplicity.  Each 128-edge group g gets a static
rank-pair count rg[g] (cross-core max); the host packs the angles as
[98, 128] blocks (rank 2p in partitions 0:49, 2p+1 in 49:98), so

    S^T[:, group g] = sum_p [W_a; W_a]^T @ aT_block(g, p)    # [8, 128]

is a plain PSUM accumulation (feature-major S).  Descending sort makes the
rank profile a staircase: ~12% padding, no overflow level, no scatter.

The apply keeps everything feature-major.  S^T round-trips through DRAM and
is re-read with a partition-broadcast DMA (each SBUF partition reads the
same DRAM bytes), giving s_bc[l, (j,e)] = S[e,j] on all 128 partitions.
Then per chunk of 1024 edges:
    z_j  = me * s_bc_j                  # DVE bf16 2x, feature-major
    outT = sum_j final_w[:,j,:] @ z_j   # PSUM accumulation over j
and outT [D, E] is written bf16; the host transposes/casts/unpermutes.
"""

import numpy as np

import concourse.bass as bass
import concourse.mybir as mybir
import concourse.tile as tile
from concourse import bacc, bass_utils

F32 = mybir.dt.float32
BF16 = mybir.dt.bfloat16
AF = mybir.ActivationFunctionType
OP = mybir.AluOpType

D = 128
A_DIM = 49
N_RBF = 6
N_BIL = 8
N_CORES = 8
AT_P = 2 * A_DIM          # 98 partitions: even rank 0:49, odd rank 49:98
AT_TILE = 4096            # aT stream tile width (cols); 32 blocks per tile


class Cfg:
    def __init__(self, e_valid, e_pad, rg):
        self.e_valid = e_valid
        self.e_pad = e_pad
        self.rg = tuple(int(r) for r in rg)      # rank-pairs per 128-edge group
        assert e_pad % 1024 == 0
        self.n_groups = e_pad // 128
        assert len(self.rg) == self.n_groups
        self.n_blocks = sum(self.rg)
        self.at_cols = ((self.n_blocks * 128 + AT_TILE - 1) // AT_TILE) * AT_TILE
        self.n_chunks = e_pad // 1024

    def key(self):
        return (self.e_valid, self.e_pad, self.rg)


def build_nc(cfg: Cfg):
    nc = bacc.Bacc(None)
    EP = cfg.e_pad
    NG = cfg.n_groups
    NC = cfg.n_chunks

    aT = nc.dram_tensor("a_t", [AT_P, cfg.at_cols], BF16, kind="ExternalInput")
    mjiT = nc.dram_tensor("mji_t", [D, EP], BF16, kind="ExternalInput")
    erbf = nc.dram_tensor("erbf_t", [N_RBF, EP], BF16, kind="ExternalInput")
    wnbr = nc.dram_tensor("w_nbr", [D, D], BF16, kind="ExternalInput")
    bnbr = nc.dram_tensor("b_nbr", [D, 1], F32, kind="ExternalInput")
    wes = nc.dram_tensor("w_e", [N_RBF, D], BF16, kind="ExternalInput")
    wa2 = nc.dram_tensor("w_a2", [AT_P, N_BIL], BF16, kind="ExternalInput")
    t2 = nc.dram_tensor("t2", [D, N_BIL * D], BF16, kind="ExternalInput")
    outd = nc.dram_tensor("out", [D, EP], BF16, kind="ExternalOutput")
    # chunk-major S^T spill, one tensor per chunk so phase B pipelines with A
    sTd = [nc.dram_tensor(f"s_t{c}", [N_BIL, 1024], BF16) for c in range(NC)]

    with tile.TileContext(nc) as tc:
        with tc.tile_pool(name="const", bufs=1) as cp:
            wa_sb = cp.tile([AT_P, N_BIL], BF16)
            nc.sync.dma_start(out=wa_sb[:], in_=wa2[:])
            wn_sb = cp.tile([D, D], BF16)
            nc.sync.dma_start(out=wn_sb[:], in_=wnbr[:])
            b_sb = cp.tile([D, 1], F32)
            nc.sync.dma_start(out=b_sb[:], in_=bnbr[:])
            we_sb = cp.tile([N_RBF, D], BF16)
            nc.sync.dma_start(out=we_sb[:], in_=wes[:])
            t2_sb = cp.tile([D, N_BIL * D], BF16)
            nc.sync.dma_start(out=t2_sb[:], in_=t2[:])

            with tc.tile_pool(name="pa", bufs=4) as pa, \
                 tc.tile_pool(name="stp", bufs=3) as stp, \
                 tc.tile_pool(name="pss", bufs=2, space="PSUM") as pss, \
                 tc.tile_pool(name="pb", bufs=2) as pb, \
                 tc.tile_pool(name="sbp", bufs=5) as sbp, \
                 tc.tile_pool(name="mjp", bufs=2) as mjp, \
                 tc.tile_pool(name="zp", bufs=3) as zp, \
                 tc.tile_pool(name="ofp", bufs=2) as ofp, \
                 tc.tile_pool(name="psmm", bufs=2, space="PSUM") as pmm, \
                 tc.tile_pool(name="psy", bufs=1, space="PSUM") as py:
                at_tiles = {}

                def at_block(b):
                    tk = b // (AT_TILE // 128)
                    if tk not in at_tiles:
                        t = pa.tile([AT_P, AT_TILE], BF16, tag="at")
                        nc.sync.dma_start(
                            out=t[:], in_=aT[:, tk * AT_TILE:(tk + 1) * AT_TILE])
                        at_tiles.clear()
                        at_tiles[tk] = t
                    off = (b % (AT_TILE // 128)) * 128
                    return at_tiles[tk][:, off:off + 128]

                blk_ctr = [0]

                def phase_a_chunk(c):
                    st = stp.tile([N_BIL, 1024], BF16, tag="st")
                    nzc = sum(1 for g in range(c * 8, c * 8 + 8)
                              if cfg.rg[g] > 0)
                    if nzc < 8:
                        nc.vector.memset(st[:], 0.0)
                    for half in range(2):
                        g0 = c * 8 + half * 4
                        nz = sum(1 for g in range(g0, g0 + 4)
                                 if cfg.rg[g] > 0)
                        if nz == 0:
                            continue
                        ps = pss.tile([N_BIL, 512], F32, tag="ps")
                        for g in range(g0, g0 + 4):
                            R = cfg.rg[g]
                            if R == 0:
                                continue
                            sl = (g - g0) * 128
                            for p in range(R):
                                nc.tensor.matmul(
                                    ps[:, sl:sl + 128], wa_sb[:],
                                    at_block(blk_ctr[0]),
                                    start=(p == 0), stop=(p == R - 1))
                                blk_ctr[0] += 1
                        nc.scalar.activation(
                            st[:, half * 512:half * 512 + nz * 128],
                            ps[:, :nz * 128], AF.Copy)
                    nc.scalar.dma_start(out=sTd[c].ap(), in_=st[:])

                state = {}

                def phase_b_chunk(c):
                    s_bc = sbp.tile([128, N_BIL * 1024], BF16,
                                    tag="sbc", name="sbc")
                    nc.sync.dma_start(
                        out=s_bc[:],
                        in_=sTd[c].ap().unsqueeze(0).broadcast_to(
                            [128, N_BIL, 1024]))
                    so = 0
                    er_sb = pb.tile([N_RBF, 1024], BF16, tag="er",
                                    name="er_sb")
                    nc.scalar.dma_start(out=er_sb[:],
                                        in_=erbf[:, c * 1024:(c + 1) * 1024])
                    te_ps = pmm.tile([128, 1024], F32, tag="mm")
                    for n in range(2):
                        nc.tensor.matmul(
                            te_ps[:, n * 512:(n + 1) * 512],
                            we_sb[:], er_sb[:, n * 512:(n + 1) * 512],
                            start=True, stop=True)
                    te_sb = pb.tile([128, 1024], BF16, tag="te")
                    nc.scalar.activation(te_sb[:], te_ps[:], AF.Copy)
                    if c % 2 == 0:
                        state["mj2"] = mjp.tile([128, 2048], BF16, tag="mj",
                                                name="mj2")
                        cend = min(c + 2, (cfg.e_valid + 1023) // 1024)
                        nc.scalar.dma_start(
                            out=state["mj2"][:, :(cend - c) * 1024],
                            in_=mjiT[:, c * 1024:cend * 1024])
                    mj = state["mj2"][:, (c % 2) * 1024:(c % 2 + 1) * 1024]
                    tm_ps = pmm.tile([128, 1024], F32, tag="mm")
                    for n in range(2):
                        nc.tensor.matmul(
                            tm_ps[:, n * 512:(n + 1) * 512],
                            wn_sb[:], mj[:, n * 512:(n + 1) * 512],
                            start=True, stop=True)
                    tm_sb = pb.tile([128, 1024], BF16, tag="tm")
                    nc.scalar.activation(tm_sb[:], tm_ps[:], AF.Silu,
                                         bias=b_sb[:, 0:1])
                    me_sb = pb.tile([128, 1024], BF16, tag="me")
                    nc.vector.tensor_mul(me_sb[:], tm_sb[:], te_sb[:])

                    z = zp.tile([128, N_BIL * 1024], BF16, tag="z")
                    for j in range(N_BIL):
                        nc.vector.tensor_mul(
                            z[:, j * 1024:(j + 1) * 1024], me_sb[:],
                            s_bc[:, j * 1024 + so:j * 1024 + so + 1024])
                    ot = py.tile([128, 1024], F32, tag="ot")
                    for h in range(2):
                        for j in range(N_BIL):
                            nc.tensor.matmul(
                                ot[:, h * 512:(h + 1) * 512],
                                t2_sb[:, j * 128:(j + 1) * 128],
                                z[:, j * 1024 + h * 512:j * 1024 + (h + 1) * 512],
                                start=(j == 0), stop=(j == N_BIL - 1))
                    if c % 2 == 0:
                        state["of2"] = ofp.tile([128, 2048], BF16, tag="of",
                                                name="of2")
                    nc.scalar.activation(
                        state["of2"][:, (c % 2) * 1024:(c % 2 + 1) * 1024],
                        ot[:], AF.Copy)
                    if c % 2 == 1 or c == (cfg.e_valid + 1023) // 1024 - 1:
                        c0 = c - (c % 2)
                        nc.scalar.dma_start(
                            out=outd[:, c0 * 1024:(c + 1) * 1024],
                            in_=state["of2"][:, :(c + 1 - c0) * 1024])

                ncb = (cfg.e_valid + 1023) // 1024
                for g in range(NG):
                    if cfg.rg[g] > 0:
                        ncb = max(ncb, g // 8 + 1)
                LAG = 4
                for c in range(ncb + LAG):
                    if c < ncb:
                        phase_a_chunk(c)
                    if c >= LAG:
                        phase_b_chunk(c - LAG)
    nc.finalize()
    return nc


# ----------------------------------------------------------------------------
# host-side sharding / unsharding
# ----------------------------------------------------------------------------

def make_cfg(kj, n_edges, ev=25_000, ep=26_624):
    n_cores = (n_edges + ev - 1) // ev
    owner = np.minimum(kj // ev, n_cores - 1)
    ng = ep // 128
    rg = np.zeros(ng, np.int64)
    for c in range(n_cores):
        loc = kj[owner == c] - c * ev
        cnt = np.bincount(loc, minlength=ev)
        s = np.zeros(ep, np.int64)
        s[:ev] = np.sort(cnt)[::-1]
        gmax = s.reshape(ng, 128).max(axis=1)
        rg = np.maximum(rg, (gmax + 1) // 2)
    return Cfg(ev, ep, tuple(int(r) for r in rg))


def prep_in_maps(cfg: Cfg, m_ji, nbr_list, angle_list, e_rbf, a_sbf, kj_idx,
                 W_nbr, b_nbr, W_e, W_a, final_w):
    del nbr_list, angle_list
    m_ji = np.asarray(m_ji, np.float32)
    e_rbf = np.asarray(e_rbf, np.float32)
    a_sbf = np.asarray(a_sbf, np.float32)
    kj = np.asarray(kj_idx).astype(np.int64)
    W_nbr = np.asarray(W_nbr, np.float32)
    b_nbr = np.asarray(b_nbr, np.float32)
    W_e = np.asarray(W_e, np.float32)
    W_a = np.asarray(W_a, np.float32)
    final_w = np.asarray(final_w, np.float32)

    n_edges = m_ji.shape[0]
    ev = cfg.e_valid
    ep = cfg.e_pad
    n_cores = (n_edges + ev - 1) // ev
    owner = np.minimum(kj // ev, n_cores - 1)

    wa2 = np.zeros((AT_P, N_BIL), np.float32)
    wa2[0:A_DIM] = W_a
    wa2[A_DIM:2 * A_DIM] = W_a
    t2 = np.ascontiguousarray(final_w.transpose(2, 1, 0).reshape(D, N_BIL * D))
    bn = np.ascontiguousarray(b_nbr.reshape(D, 1))

    in_maps = []
    perms = []
    for c in range(n_cores):
        sel = np.nonzero(owner == c)[0]
        loc = kj[sel] - c * ev
        cnt = np.bincount(loc, minlength=ev)
        edge_order = np.argsort(-cnt, kind="stable")     # slot -> local edge
        slot_of_edge = np.empty(ev, np.int64)
        slot_of_edge[edge_order] = np.arange(ev)
        ang_slot = slot_of_edge[loc]
        order = np.argsort(ang_slot, kind="stable")
        rows = sel[order]                 # a_sbf row per (slot-sorted) token
        cnt_slot = np.bincount(ang_slot, minlength=ep)
        starts = np.concatenate([[0], np.cumsum(cnt_slot)])

        at = np.zeros((AT_P, cfg.at_cols), np.float32)
        col = 0
        for g in range(cfg.n_groups):
            sl = np.arange(g * 128, (g + 1) * 128)
            csl = cnt_slot[sl]
            for p in range(cfg.rg[g]):
                for half, r in ((0, 2 * p), (1, 2 * p + 1)):
                    has = np.nonzero(csl > r)[0]
                    if len(has):
                        tok = starts[sl[has]] + r
                        at[half * A_DIM:(half + 1) * A_DIM,
                           col + has] = a_sbf[rows[tok]].T
                col += 128
        assert col == cfg.n_blocks * 128

        e0, e1 = c * ev, min((c + 1) * ev, n_edges)
        mjiT = np.zeros((D, ep), np.float32)
        mjiT[:, :e1 - e0] = m_ji[e0:e1][edge_order[:e1 - e0]].T
        erbfT = np.zeros((N_RBF, ep), np.float32)
        erbfT[:, :e1 - e0] = e_rbf[e0:e1][edge_order[:e1 - e0]].T

        bf = mybir.dt.np(BF16)
        im = {
            "a_t": at.astype(bf), "mji_t": mjiT.astype(bf),
            "erbf_t": erbfT.astype(bf), "w_nbr": W_nbr.astype(bf),
            "b_nbr": bn, "w_e": W_e.astype(bf), "w_a2": wa2.astype(bf),
            "t2": t2.astype(bf),
        }
        in_maps.append(im)
        perms.append(edge_order)
    return in_maps, perms


def gather_output(cfg: Cfg, results, perms, n_edges):
    ev = cfg.e_valid
    out = np.empty((n_edges, D), np.float32)
    for c, r in enumerate(results):
        e0, e1 = c * ev, min((c + 1) * ev, n_edges)
        dev = np.asarray(r["out"]).astype(np.float32)       # [D, EP]
        out[e0 + perms[c][:e1 - e0]] = dev[:, :e1 - e0].T
    return out


_NC_CACHE = {}


def run_on_hw(inputs, cfg=None, trace=False, trace_cores=None):
    kj = np.asarray(inputs["kj_idx"]).astype(np.int64)
    if cfg is None:
        cfg = make_cfg(kj, inputs["m_ji"].shape[0])
    key = cfg.key()
    if key not in _NC_CACHE:
        _NC_CACHE[key] = build_nc(cfg)
    nc = _NC_CACHE[key]
    in_maps, perms = prep_in_maps(cfg, **inputs)
    res = bass_utils.run_bass_kernel_spmd(
        nc, in_maps, core_ids=list(range(len(in_maps))),
        trace=trace, trace_cores=trace_cores)
    out = gather_output(cfg, res.results, perms, inputs["m_ji"].shape[0])
    return out, res


def kernel(**inputs) -> np.ndarray:
    out, _ = run_on_hw(inputs)
    return out


# revision 35
# speedup vs baseline: 1.0004x; 1.0004x over previous
"""Trainium2 Bass kernel for the DimeNet-style directed-message block.

Reference computation (W = n_angles, E = n_edges, D = 128, A = 49, J = 8):
    m_kj     = m_ji[kj_idx]                          # [W, D]
    transf_m = silu(m_kj @ W_nbr + b_nbr)            # [W, D]
    transf_e = e_rbf[kj_idx] @ W_e                   # [W, D]
    m_and_e  = transf_m * transf_e                   # [W, D]
    transf_a = a_sbf @ W_a                           # [W, J]
    out[w,i] = sum_{j,l} transf_a[w,j] m_and_e[w,l] final_w[i,j,l]
    final    = segment_sum(out, kj_idx, E)           # [E, D]

Algebraic refactor: the segment sum commutes through the bilinear form:
    me       = silu(m_ji @ W_nbr + b) * (e_rbf @ W_e)        # [E, D]
    S        = segment_sum(a_sbf @ W_a, kj_idx, E)           # [E, J]
    final[e] = sum_j S[e,j] * (me[e] @ final_w[:,j,:].T)     # [E, D]

S without scatter: edges are sharded contiguously (25000/core, angles
binned by owner core kj // 25000) and permuted within the core by
descending angle multiplicity.  Each 128-edge group g gets a static
rank-pair count rg[g] (cross-core max); the host packs the angles as
[98, 128] blocks (rank 2p in partitions 0:49, 2p+1 in 49:98), so

    S^T[:, group g] = sum_p [W_a; W_a]^T @ aT_block(g, p)    # [8, 128]

is a plain PSUM accumulation (feature-major S).  Descending sort makes the
rank profile a staircase: ~12% padding, no overflow level, no scatter.

The apply keeps everything feature-major.  S^T round-trips through DRAM and
is re-read with a partition-broadcast DMA (each SBUF partition reads the
same DRAM bytes), giving s_bc[l, (j,e)] = S[e,j] on all 128 partitions.
Then per chunk of 1024 edges:
    z_j  = me * s_bc_j                  # DVE bf16 2x, feature-major
    outT = sum_j final_w[:,j,:] @ z_j   # PSUM accumulation over j
and outT [D, E] is written bf16; the host transposes/casts/unpermutes.
"""

import numpy as np

import concourse.bass as bass
import concourse.mybir as mybir
import concourse.tile as tile
from concourse import bacc, bass_utils

F32 = mybir.dt.float32
BF16 = mybir.dt.bfloat16
AF = mybir.ActivationFunctionType
OP = mybir.AluOpType

D = 128
A_DIM = 49
N_RBF = 6
N_BIL = 8
N_CORES = 8
AT_P = 2 * A_DIM          # 98 partitions: even rank 0:49, odd rank 49:98
AT_TILE = 4096            # aT stream tile width (cols); 32 blocks per tile


class Cfg:
    def __init__(self, e_valid, e_pad, rg):
        self.e_valid = e_valid
        self.e_pad = e_pad
        self.rg = tuple(int(r) for r in rg)      # rank-pairs per 128-edge group
        assert e_pad % 1024 == 0
        self.n_groups = e_pad // 128
        assert len(self.rg) == self.n_groups
        self.n_blocks = sum(self.rg)
        self.at_cols = ((self.n_blocks * 128 + AT_TILE - 1) // AT_TILE) * AT_TILE
        self.n_chunks = e_pad // 1024

    def key(self):
        return (self.e_valid, self.e_pad, self.rg)


def build_nc(cfg: Cfg):
    nc = bacc.Bacc(None)
    EP = cfg.e_pad
    NG = cfg.n_groups
    NC = cfg.n_chunks

    aT = nc.dram_tensor("a_t", [AT_P, cfg.at_cols], BF16, kind="ExternalInput")
    mjiT = nc.dram_tensor("mji_t", [D, EP], BF16, kind="ExternalInput")
    erbf = nc.dram_tensor("erbf_t", [N_RBF, EP], BF16, kind="ExternalInput")
    wnbr = nc.dram_tensor("w_nbr", [D, D], BF16, kind="ExternalInput")
    bnbr = nc.dram_tensor("b_nbr", [D, 1], F32, kind="ExternalInput")
    wes = nc.dram_tensor("w_e", [N_RBF, D], BF16, kind="ExternalInput")
    wa2 = nc.dram_tensor("w_a2", [AT_P, N_BIL], BF16, kind="ExternalInput")
    t2 = nc.dram_tensor("t2", [D, N_BIL * D], BF16, kind="ExternalInput")
    outd = nc.dram_tensor("out", [D, EP], BF16, kind="ExternalOutput")
    # chunk-major S^T spill, one tensor per chunk so phase B pipelines with A
    sTd = [nc.dram_tensor(f"s_t{c}", [N_BIL, 1024], BF16) for c in range(NC)]

    with tile.TileContext(nc) as tc:
        with tc.tile_pool(name="const", bufs=1) as cp:
            wa_sb = cp.tile([AT_P, N_BIL], BF16)
            nc.sync.dma_start(out=wa_sb[:], in_=wa2[:])
            wn_sb = cp.tile([D, D], BF16)
            nc.sync.dma_start(out=wn_sb[:], in_=wnbr[:])
            b_sb = cp.tile([D, 1], F32)
            nc.sync.dma_start(out=b_sb[:], in_=bnbr[:])
            we_sb = cp.tile([N_RBF, D], BF16)
            nc.sync.dma_start(out=we_sb[:], in_=wes[:])
            t2_sb = cp.tile([D, N_BIL * D], BF16)
            nc.sync.dma_start(out=t2_sb[:], in_=t2[:])

            with tc.tile_pool(name="pa", bufs=4) as pa, \
                 tc.tile_pool(name="stp", bufs=3) as stp, \
                 tc.tile_pool(name="pss", bufs=2, space="PSUM") as pss, \
                 tc.tile_pool(name="pb", bufs=2) as pb, \
                 tc.tile_pool(name="sbp", bufs=4) as sbp, \
                 tc.tile_pool(name="mjp", bufs=2) as mjp, \
                 tc.tile_pool(name="zp", bufs=3) as zp, \
                 tc.tile_pool(name="ofp", bufs=2) as ofp, \
                 tc.tile_pool(name="psmm", bufs=2, space="PSUM") as pmm, \
                 tc.tile_pool(name="psy", bufs=1, space="PSUM") as py:
                at_tiles = {}

                def at_block(b):
                    tk = b // (AT_TILE // 128)
                    if tk not in at_tiles:
                        t = pa.tile([AT_P, AT_TILE], BF16, tag="at")
                        nc.sync.dma_start(
                            out=t[:], in_=aT[:, tk * AT_TILE:(tk + 1) * AT_TILE])
                        at_tiles.clear()
                        at_tiles[tk] = t
                    off = (b % (AT_TILE // 128)) * 128
                    return at_tiles[tk][:, off:off + 128]

                blk_ctr = [0]

                def phase_a_chunk(c):
                    st = stp.tile([N_BIL, 1024], BF16, tag="st")
                    nzc = sum(1 for g in range(c * 8, c * 8 + 8)
                              if cfg.rg[g] > 0)
                    if nzc < 8:
                        nc.vector.memset(st[:], 0.0)
                    for half in range(2):
                        g0 = c * 8 + half * 4
                        nz = sum(1 for g in range(g0, g0 + 4)
                                 if cfg.rg[g] > 0)
                        if nz == 0:
                            continue
                        ps = pss.tile([N_BIL, 512], F32, tag="ps")
                        for g in range(g0, g0 + 4):
                            R = cfg.rg[g]
                            if R == 0:
                                continue
                            sl = (g - g0) * 128
                            for p in range(R):
                                nc.tensor.matmul(
                                    ps[:, sl:sl + 128], wa_sb[:],
                                    at_block(blk_ctr[0]),
                                    start=(p == 0), stop=(p == R - 1))
                                blk_ctr[0] += 1
                        nc.scalar.activation(
                            st[:, half * 512:half * 512 + nz * 128],
                            ps[:, :nz * 128], AF.Copy)
                    nc.scalar.dma_start(out=sTd[c].ap(), in_=st[:])

                state = {}

                def phase_b_chunk(c):
                    s_bc = sbp.tile([128, N_BIL * 1024], BF16,
                                    tag="sbc", name="sbc")
                    nc.sync.dma_start(
                        out=s_bc[:],
                        in_=sTd[c].ap().unsqueeze(0).broadcast_to(
                            [128, N_BIL, 1024]))
                    so = 0
                    er_sb = pb.tile([N_RBF, 1024], BF16, tag="er",
                                    name="er_sb")
                    nc.scalar.dma_start(out=er_sb[:],
                                        in_=erbf[:, c * 1024:(c + 1) * 1024])
                    te_ps = pmm.tile([128, 1024], F32, tag="mm")
                    for n in range(2):
                        nc.tensor.matmul(
                            te_ps[:, n * 512:(n + 1) * 512],
                            we_sb[:], er_sb[:, n * 512:(n + 1) * 512],
                            start=True, stop=True)
                    te_sb = pb.tile([128, 1024], BF16, tag="te")
                    nc.scalar.activation(te_sb[:], te_ps[:], AF.Copy)
                    if c % 2 == 0:
                        state["mj2"] = mjp.tile([128, 2048], BF16, tag="mj",
                                                name="mj2")
                        cend = min(c + 2, (cfg.e_valid + 1023) // 1024)
                        nc.scalar.dma_start(
                            out=state["mj2"][:, :(cend - c) * 1024],
                            in_=mjiT[:, c * 1024:cend * 1024])
                    mj = state["mj2"][:, (c % 2) * 1024:(c % 2 + 1) * 1024]
                    tm_ps = pmm.tile([128, 1024], F32, tag="mm")
                    for n in range(2):
                        nc.tensor.matmul(
                            tm_ps[:, n * 512:(n + 1) * 512],
                            wn_sb[:], mj[:, n * 512:(n + 1) * 512],
                            start=True, stop=True)
                    tm_sb = pb.tile([128, 1024], BF16, tag="tm")
                    nc.scalar.activation(tm_sb[:], tm_ps[:], AF.Silu,
                                         bias=b_sb[:, 0:1])
                    me_sb = pb.tile([128, 1024], BF16, tag="me")
                    nc.vector.tensor_mul(me_sb[:], tm_sb[:], te_sb[:])

                    z = zp.tile([128, N_BIL * 1024], BF16, tag="z")
                    for j in range(N_BIL):
                        nc.vector.tensor_mul(
                            z[:, j * 1024:(j + 1) * 1024], me_sb[:],
                            s_bc[:, j * 1024 + so:j * 1024 + so + 1024])
                    ot = py.tile([128, 1024], F32, tag="ot")
                    for h in range(2):
                        for j in range(N_BIL):
                            nc.tensor.matmul(
                                ot[:, h * 512:(h + 1) * 512],
                                t2_sb[:, j * 128:(j + 1) * 128],
                                z[:, j * 1024 + h * 512:j * 1024 + (h + 1) * 512],
                                start=(j == 0), stop=(j == N_BIL - 1))
                    if c % 2 == 0:
                        state["of2"] = ofp.tile([128, 2048], BF16, tag="of",
                                                name="of2")
                    nc.scalar.activation(
                        state["of2"][:, (c % 2) * 1024:(c % 2 + 1) * 1024],
                        ot[:], AF.Copy)
                    if c % 2 == 1 or c == (cfg.e_valid + 1023) // 1024 - 1:
                        c0 = c - (c % 2)
                        nc.scalar.dma_start(
                            out=outd[:, c0 * 1024:(c + 1) * 1024],
                            in_=state["of2"][:, :(c + 1 - c0) * 1024])

                ncb = (cfg.e_valid + 1023) // 1024
                for g in range(NG):
                    if cfg.rg[g] > 0:
                        ncb = max(ncb, g // 8 + 1)
                LAG = 3
                for c in range(ncb + LAG):
                    if c < ncb:
                        phase_a_chunk(c)
                    if c >= LAG:
                        phase_b_chunk(c - LAG)
    nc.finalize()
    return nc


# ----------------------------------------------------------------------------
# host-side sharding / unsharding
# ----------------------------------------------------------------------------

def make_cfg(kj, n_edges, ev=25_000, ep=26_624):
    n_cores = (n_edges + ev - 1) // ev
    owner = np.minimum(kj // ev, n_cores - 1)
    ng = ep // 128
    rg = np.zeros(ng, np.int64)
    for c in range(n_cores):
        loc = kj[owner == c] - c * ev
        cnt = np.bincount(loc, minlength=ev)
        s = np.zeros(ep, np.int64)
        s[:ev] = np.sort(cnt)[::-1]
        gmax = s.reshape(ng, 128).max(axis=1)
        rg = np.maximum(rg, (gmax + 1) // 2)
    return Cfg(ev, ep, tuple(int(r) for r in rg))


def prep_in_maps(cfg: Cfg, m_ji, nbr_list, angle_list, e_rbf, a_sbf, kj_idx,
                 W_nbr, b_nbr, W_e, W_a, final_w):
    del nbr_list, angle_list
    m_ji = np.asarray(m_ji, np.float32)
    e_rbf = np.asarray(e_rbf, np.float32)
    a_sbf = np.asarray(a_sbf, np.float32)
    kj = np.asarray(kj_idx).astype(np.int64)
    W_nbr = np.asarray(W_nbr, np.float32)
    b_nbr = np.asarray(b_nbr, np.float32)
    W_e = np.asarray(W_e, np.float32)
    W_a = np.asarray(W_a, np.float32)
    final_w = np.asarray(final_w, np.float32)

    n_edges = m_ji.shape[0]
    ev = cfg.e_valid
    ep = cfg.e_pad
    n_cores = (n_edges + ev - 1) // ev
    owner = np.minimum(kj // ev, n_cores - 1)

    wa2 = np.zeros((AT_P, N_BIL), np.float32)
    wa2[0:A_DIM] = W_a
    wa2[A_DIM:2 * A_DIM] = W_a
    t2 = np.ascontiguousarray(final_w.transpose(2, 1, 0).reshape(D, N_BIL * D))
    bn = np.ascontiguousarray(b_nbr.reshape(D, 1))

    in_maps = []
    perms = []
    for c in range(n_cores):
        sel = np.nonzero(owner == c)[0]
        loc = kj[sel] - c * ev
        cnt = np.bincount(loc, minlength=ev)
        edge_order = np.argsort(-cnt, kind="stable")     # slot -> local edge
        slot_of_edge = np.empty(ev, np.int64)
        slot_of_edge[edge_order] = np.arange(ev)
        ang_slot = slot_of_edge[loc]
        order = np.argsort(ang_slot, kind="stable")
        rows = sel[order]                 # a_sbf row per (slot-sorted) token
        cnt_slot = np.bincount(ang_slot, minlength=ep)
        starts = np.concatenate([[0], np.cumsum(cnt_slot)])

        at = np.zeros((AT_P, cfg.at_cols), np.float32)
        col = 0
        for g in range(cfg.n_groups):
            sl = np.arange(g * 128, (g + 1) * 128)
            csl = cnt_slot[sl]
            for p in range(cfg.rg[g]):
                for half, r in ((0, 2 * p), (1, 2 * p + 1)):
                    has = np.nonzero(csl > r)[0]
                    if len(has):
                        tok = starts[sl[has]] + r
                        at[half * A_DIM:(half + 1) * A_DIM,
                           col + has] = a_sbf[rows[tok]].T
                col += 128
        assert col == cfg.n_blocks * 128

        e0, e1 = c * ev, min((c + 1) * ev, n_edges)
        mjiT = np.zeros((D, ep), np.float32)
        mjiT[:, :e1 - e0] = m_ji[e0:e1][edge_order[:e1 - e0]].T
        erbfT = np.zeros((N_RBF, ep), np.float32)
        erbfT[:, :e1 - e0] = e_rbf[e0:e1][edge_order[:e1 - e0]].T

        bf = mybir.dt.np(BF16)
        im = {
            "a_t": at.astype(bf), "mji_t": mjiT.astype(bf),
            "erbf_t": erbfT.astype(bf), "w_nbr": W_nbr.astype(bf),
            "b_nbr": bn, "w_e": W_e.astype(bf), "w_a2": wa2.astype(bf),
            "t2": t2.astype(bf),
        }
        in_maps.append(im)
        perms.append(edge_order)
    return in_maps, perms


def gather_output(cfg: Cfg, results, perms, n_edges):
    ev = cfg.e_valid
    out = np.empty((n_edges, D), np.float32)
    for c, r in enumerate(results):
        e0, e1 = c * ev, min((c + 1) * ev, n_edges)
        dev = np.asarray(r["out"]).astype(np.float32)       # [D, EP]
        out[e0 + perms[c][:e1 - e0]] = dev[:, :e1 - e0].T
    return out


_NC_CACHE = {}


def run_on_hw(inputs, cfg=None, trace=False, trace_cores=None):
    kj = np.asarray(inputs["kj_idx"]).astype(np.int64)
    if cfg is None:
        cfg = make_cfg(kj, inputs["m_ji"].shape[0])
    key = cfg.key()
    if key not in _NC_CACHE:
        _NC_CACHE[key] = build_nc(cfg)
    nc = _NC_CACHE[key]
    in_maps, perms = prep_in_maps(cfg, **inputs)
    res = bass_utils.run_bass_kernel_spmd(
        nc, in_maps, core_ids=list(range(len(in_maps))),
        trace=trace, trace_cores=trace_cores)
    out = gather_output(cfg, res.results, perms, inputs["m_ji"].shape[0])
    return out, res


def kernel(**inputs) -> np.ndarray:
    out, _ = run_on_hw(inputs)
    return out
